# revision 1
# baseline (speedup 1.0000x reference)
"""Causal multi-head attention (B=2,T=2048,C=1024,H=16,Ca=64) on 8 trn2 cores.

Sharding: the 32 (batch, head) pairs are split across 8 cores — core c gets
batch b = c//4 and heads [4g, 4g+4) where g = c%4.  Each core computes its
heads' attention plus the partial output projection through its 256-row slice
of w_o; the host sums the 4 partials per batch.

Per-core layouts (everything keeps the contraction dim on partitions):
  xT   [8,128,2048]  x[b].T c-chunked
  wq/wk[2,8,128,128] per head-pair, per c-chunk, cols = [h0 64 | h1 64]
  wv   [8,128,256]   4 heads concatenated
  wo   [2,128,1024]  rows 256g..256g+256 of w_o, c_local-chunked
  out  [16,128,1024] partial output, t-blocked

On-chip: Q^T,K^T [128(2 heads),2048]; V natural [s,a] with a ones column
appended so the A@V matmul also emits the softmax row-sums l[t]; scores are
computed transposed (S^T[s,t]) so softmax needs no cross-partition reduction
and no max-subtraction (logits are bounded: |s*scale| < ~4).
"""

import math
import sys

import numpy as np

for _p in ("/opt/trn_rl_repo",):
    if _p not in sys.path:
        sys.path.insert(0, _p)

import concourse.bass as bass
from concourse import bacc
import concourse.mybir as mybir
from concourse.bass import ts
from concourse.tile import TileContext
from concourse.bass_utils import run_bass_kernel_spmd
from contextlib import ExitStack

F32 = mybir.dt.float32
F32R = mybir.dt.float32r
AF = mybir.ActivationFunctionType

B, T, C = 2, 2048, 1024
H, CA = 16, 64
SCALE = 1.0 / math.sqrt(CA)
NCORES = 8
HPC = 4          # heads per core
TB = T // 128    # 16 t-blocks of 128
TC = T // 512    # 4 t-chunks of 512
CK = C // 128    # 8 c-chunks




def build_nc():
    nc = bacc.Bacc()
    xT = nc.declare_dram_parameter("xT", [CK, 128, T], F32R, isOutput=False)
    wq = nc.declare_dram_parameter("wq", [2, CK, 128, 128], F32R, isOutput=False)
    wk = nc.declare_dram_parameter("wk", [2, CK, 128, 128], F32R, isOutput=False)
    wv = nc.declare_dram_parameter("wv", [CK, 128, 2 * 128], F32R, isOutput=False)
    wo = nc.declare_dram_parameter("wo", [2, 128, C], F32R, isOutput=False)
    mask_d = nc.declare_dram_parameter("mask", [128, 4, 512], F32R, isOutput=False)
    ones_d = nc.declare_dram_parameter("ones", [128, 64], F32R, isOutput=False)
    out = nc.declare_dram_parameter("out", [TB, 128, C], F32, isOutput=True)

    with TileContext(nc) as tc, ExitStack() as ctx:
        const = ctx.enter_context(tc.tile_pool(name="const", bufs=1))
        persist = ctx.enter_context(tc.tile_pool(name="persist", bufs=1))

        # 0/1 causal masks for the 4 diagonal-band shifts (S^T layout [s,t]):
        # keep (1.0) where 128*d + p <= f, else 0.  Host-computed.
        mask = const.tile([128, 4, 512], F32R)
        nc.scalar.dma_start(mask[:], mask_d[:])
        ones_sb = const.tile([128, 64], F32R)
        nc.gpsimd.dma_start(ones_sb[:], ones_d[:])
        ones1 = ones_sb[0:1, :]

        q_sb = [persist.tile([128, T], F32R, tag=f"q{p}", name=f"q{p}") for p in range(2)]
        k_sb = [persist.tile([128, T], F32R, tag=f"k{p}", name=f"k{p}") for p in range(2)]
        # V natural [s,a] per head, t-blocked, with ones column at a=64
        v_sb = persist.tile([128, HPC, TB, 65], F32R, tag="v")
        nc.sync.dma_start(
            v_sb[:, :, :, 64],
            ones_d[:].rearrange("p (h b) -> p h b", h=HPC),
        )
        y_sb = [persist.tile([128, T], F32R, tag=f"y{p}", name=f"y{p}") for p in range(2)]
        wo_sb = persist.tile([128, 2, C], F32R, tag="wo")
        for cl in range(2):
            nc.gpsimd.dma_start(wo_sb[:, cl, :], wo[cl])

        # ---------------- Phase B/C: projections ----------------
        with ExitStack() as pbc:
            xw = pbc.enter_context(tc.tile_pool(name="xw", bufs=1))
            ps_qk = pbc.enter_context(tc.tile_pool(name="ps_qk", bufs=4, space="PSUM"))
            ps_v = pbc.enter_context(tc.tile_pool(name="ps_v", bufs=3, space="PSUM"))

            xT_sb = xw.tile([128, CK, T], F32R, tag="xT")
            wq_sb = xw.tile([128, 2, CK, 128], F32R, tag="wq")
            wk_sb = xw.tile([128, 2, CK, 128], F32R, tag="wk")
            wv_sb = xw.tile([128, CK, 256], F32R, tag="wv")
            # weights for pair 0 first (first matmuls need them), x chunks
            # round-robined over issuing engines so queues run in parallel
            engs = [nc.sync, nc.scalar, nc.gpsimd]
            nc.sync.dma_start(xT_sb[:, 0, :], xT[0])
            nc.scalar.dma_start(wq_sb[:, 0, 0, :], wq[0, 0])
            for ck in range(1, CK):
                engs[ck % 3].dma_start(wq_sb[:, 0, ck, :], wq[0, ck])
            for ck in range(1, CK):
                engs[ck % 3].dma_start(xT_sb[:, ck, :], xT[ck])
            for ck in range(CK):
                engs[(ck + 1) % 3].dma_start(wk_sb[:, 0, ck, :], wk[0, ck])
                engs[(ck + 2) % 3].dma_start(wq_sb[:, 1, ck, :], wq[1, ck])
                engs[ck % 3].dma_start(wk_sb[:, 1, ck, :], wk[1, ck])
                engs[(ck + 1) % 3].dma_start(wv_sb[:, ck, :], wv[ck])

            # Q^T / K^T: [128(2 heads), T]
            for p in range(2):
                for w_s, dst in ((wq_sb, q_sb), (wk_sb, k_sb)):
                    pst = [ps_qk.tile([128, 512], F32, tag="qk", name="qkps") for _ in range(TC)]
                    for ck in range(CK):
                        for tcn in range(TC):
                            nc.tensor.matmul(
                                pst[tcn][:],
                                lhsT=(w_s[:, p, ck, :]),
                                rhs=(xT_sb[:, ck, ts(tcn, 512)]),
                                start=(ck == 0), stop=(ck == CK - 1),
                            )
                    for tcn in range(TC):
                        nc.vector.tensor_copy(dst[p][:, ts(tcn, 512)], pst[tcn][:])

            # V natural: [s(=t) blocks, 4*64]
            for tb in range(TB):
                vps = ps_v.tile([128, 256], F32, tag="v")
                for ck in range(CK):
                    nc.tensor.matmul(
                        vps[:],
                        lhsT=(xT_sb[:, ck, ts(tb, 128)]),
                        rhs=(wv_sb[:, ck, :]),
                        start=(ck == 0), stop=(ck == CK - 1),
                    )
                nc.vector.tensor_copy(
                    v_sb[:, :, tb, 0:64],
                    vps[:].rearrange("p (h a) -> p h a", h=HPC),
                )

        # ---------------- Phase D + E: attention and output projection ----
        # tcn-outer so the projection for finished t-chunks overlaps attention
        with ExitStack() as pd:
            pp = pd.enter_context(tc.tile_pool(name="pp", bufs=10))
            sm = pd.enter_context(tc.tile_pool(name="sm", bufs=4))
            ob = pd.enter_context(tc.tile_pool(name="ob", bufs=3))
            ps_s = pd.enter_context(tc.tile_pool(name="ps_s", bufs=2, space="PSUM"))
            ps_y = pd.enter_context(tc.tile_pool(name="ps_y", bufs=2, space="PSUM"))
            ps_o = pd.enter_context(tc.tile_pool(name="ps_o", bufs=2, space="PSUM"))

            def proj_block(tb):
                ot = ob.tile([128, C], F32, tag="o", name="ot")
                for cc in range(2):
                    ops_ = ps_o.tile([128, 512], F32, tag="o", name="ops")
                    for cl in range(2):
                        nc.tensor.matmul(
                            ops_[:],
                            lhsT=(y_sb[cl][:, ts(tb, 128)]),
                            rhs=(wo_sb[:, cl, ts(cc, 512)]),
                            start=(cl == 0), stop=(cl == 1),
                        )
                    nc.vector.tensor_copy(ot[:, ts(cc, 512)], ops_[:])
                nc.sync.dma_start(out[tb], ot[:])

            for tcn in range(TC):
                nsb = 4 * tcn + 4
                for p in range(2):
                    for hl in range(2):
                        h = 2 * p + hl
                        b0 = 64 * hl
                        yps = ps_y.tile([128, 512], F32, tag="y", name="yps")
                        for sb2 in range(0, nsb, 2):
                            sps = ps_s.tile([128, 1024], F32, tag="s", name="sps")
                            for j in range(2):
                                nc.tensor.matmul(
                                    sps[:, ts(j, 512)],
                                    lhsT=(k_sb[p][b0:b0 + 64, ts(sb2 + j, 128)]),
                                    rhs=(q_sb[p][b0:b0 + 64, ts(tcn, 512)]),
                                    start=True, stop=True,
                                )
                            pb = pp.tile([128, 1024], F32R, tag="pb", name="pb")
                            nc.scalar.activation(pb[:], sps[:], AF.Exp, scale=SCALE)
                            for j in range(2):
                                d = sb2 + j - 4 * tcn
                                if d >= 0:
                                    w = 128 * (d + 1)
                                    o = 512 * j
                                    nc.vector.tensor_mul(
                                        pb[:, o:o + w], pb[:, o:o + w],
                                        mask[:, d, :w])
                            for j in range(2):
                                nc.tensor.matmul(
                                    yps[0:65, :],
                                    lhsT=(v_sb[:, h, sb2 + j, :]),
                                    rhs=(pb[:, ts(j, 512)]),
                                    start=(sb2 + j == 0), stop=(sb2 + j == nsb - 1),
                                )
                        # normalize: y /= l (l = row 64 of yps)
                        lrow = sm.tile([1, 512], F32R, tag="l", name="lrow")
                        nc.vector.tensor_copy(lrow[:], yps[64:65, :])
                        bps = ps_o.tile([128, 512], F32, tag="o", name="bps")
                        nc.tensor.matmul(
                            bps[0:64, :], lhsT=(ones1[:]), rhs=(lrow[:]),
                            start=True, stop=True,
                        )
                        rb = sm.tile([64, 512], F32, tag="r", name="rb")
                        nc.vector.reciprocal(rb[:], bps[0:64, :])
                        nc.vector.tensor_mul(
                            y_sb[p][b0:b0 + 64, ts(tcn, 512)],
                            yps[0:64, :], rb[:],
                        )
                # project the 4 t-blocks of this finished chunk
                for tb in range(4 * tcn, 4 * tcn + 4):
                    proj_block(tb)

    nc.compile()
    return nc


_NC = None


def _get_nc():
    global _NC
    if _NC is None:
        _NC = build_nc()
    return _NC


def _mask_arr():
    p = np.arange(128)[:, None, None]
    d = np.arange(4)[None, :, None]
    f = np.arange(512)[None, None, :]
    return np.ascontiguousarray((128 * d + p <= f).astype(np.float32))


def make_in_maps(x, w_q, w_k, w_v, w_o):
    x = np.asarray(x, dtype=np.float32)
    w_q = np.asarray(w_q, dtype=np.float32)
    w_k = np.asarray(w_k, dtype=np.float32)
    w_v = np.asarray(w_v, dtype=np.float32)
    w_o = np.asarray(w_o, dtype=np.float32)
    in_maps = []
    for c in range(NCORES):
        b, g = c // 4, c % 4
        hs = [4 * g + i for i in range(HPC)]
        xT = np.ascontiguousarray(x[b].T).reshape(CK, 128, T)
        wq_a = np.stack([
            np.concatenate([w_q[hs[2 * p]], w_q[hs[2 * p + 1]]], axis=1).reshape(CK, 128, 128)
            for p in range(2)
        ])
        wk_a = np.stack([
            np.concatenate([w_k[hs[2 * p]], w_k[hs[2 * p + 1]]], axis=1).reshape(CK, 128, 128)
            for p in range(2)
        ])
        wv_a = np.concatenate([w_v[h] for h in hs], axis=1).reshape(CK, 128, 256)
        wo_a = w_o[256 * g:256 * (g + 1)].reshape(2, 128, C)
        in_maps.append(dict(
            mask=_mask_arr(),
            ones=np.ones((128, 64), np.float32),
            xT=np.ascontiguousarray(xT),
            wq=np.ascontiguousarray(wq_a),
            wk=np.ascontiguousarray(wk_a),
            wv=np.ascontiguousarray(wv_a),
            wo=np.ascontiguousarray(wo_a),
        ))
    return in_maps


def gather_out(results):
    acc = [np.zeros((T, C), np.float64) for _ in range(B)]
    for c in range(NCORES):
        acc[c // 4] += results[c]["out"].reshape(T, C).astype(np.float64)
    return np.stack([a.astype(np.float32) for a in acc])


def run(x, w_q, w_k, w_v, w_o, trace=False, **spmd_kwargs):
    nc = _get_nc()
    in_maps = make_in_maps(x, w_q, w_k, w_v, w_o)
    res = run_bass_kernel_spmd(nc, in_maps, list(range(NCORES)), trace=trace,
                               **spmd_kwargs)
    return gather_out(res.results), res


def kernel(x, w_q, w_k, w_v, w_o):
    out, _ = run(x, w_q, w_k, w_v, w_o)
    return out



# revision 18
# speedup vs baseline: 1.4698x; 1.4698x over previous
"""Causal multi-head attention (B=2,T=2048,C=1024,H=16,Ca=64) on 8 trn2 cores.

Sharding: the 32 (batch, head) pairs are split across 8 cores - core c gets
batch b = c//4 and heads [4g, 4g+4) where g = c%4.  Each core computes its
heads' attention plus the partial output projection through its 256-row slice
of w_o; the host sums the 4 partials per batch.

v2 design (all matmul operands bf16, fp32 PSUM accumulation):
  - Q^T/K^T per head pair [128(2x64a), T]; V natural [s, a] with a ones
    column at a=64 so the A@V matmul also emits softmax row sums.
  - Scores S^T[s,t] per s-block at t-chunk (512) granularity, diagonal
    blocks width-trimmed; exp on ACT; causal triangle mask multiplied on
    the (single) boundary 128x128 block of each diagonal s-block.
  - A@V transposed: lhsT = p-block [s,128t], rhs = v [s,65] -> y natural
    [t, 64+l].  Softmax division via per-partition reciprocal+scalar-mul
    (l is a per-partition scalar in this layout).  Pair tiles [t,128c]
    are transposed back to y^T via DMA-transpose for the w_o projection.
  - Hand-interleaved emission order keeps PE fed while ACT (exp) streams.
"""

import math
import sys

import numpy as np

for _p in ("/opt/trn_rl_repo",):
    if _p not in sys.path:
        sys.path.insert(0, _p)

import ml_dtypes
import concourse.bass as bass
from concourse import bacc
import concourse.mybir as mybir
from concourse.bass import ts
from concourse.tile import TileContext
from concourse.bass_utils import run_bass_kernel_spmd
from contextlib import ExitStack

F32 = mybir.dt.float32
BF16 = mybir.dt.bfloat16
AF = mybir.ActivationFunctionType
BF = ml_dtypes.bfloat16

B, T, C = 2, 2048, 1024
H, CA = 16, 64
SCALE = 1.0 / math.sqrt(CA)
NCORES = 8
HPC = 4          # heads per core
TB = T // 128    # 16 t-blocks of 128
TC = T // 512    # 4 t-chunks of 512
CK = C // 128    # 8 c-chunks


def build_nc():
    nc = bacc.Bacc()
    xT = nc.declare_dram_parameter("xT", [CK, 128, T], BF16, isOutput=False)
    # weights pre-transposed on host so partition dim is outermost-contiguous
    wq = nc.declare_dram_parameter("wq", [2, 128, CK, 128], BF16, isOutput=False)
    wk = nc.declare_dram_parameter("wk", [2, 128, CK, 128], BF16, isOutput=False)
    wv = nc.declare_dram_parameter("wv", [128, CK, 256], BF16, isOutput=False)
    wo = nc.declare_dram_parameter("wo", [2, 128, C], BF16, isOutput=False)
    mask_d = nc.declare_dram_parameter("mask", [128, 256], BF16, isOutput=False)
    ident_d = nc.declare_dram_parameter("ident", [128, 128], BF16, isOutput=False)
    out = nc.declare_dram_parameter("out", [TB, 128, C], BF16, isOutput=True)

    with TileContext(nc) as tc, ExitStack() as ctx:
        persist = ctx.enter_context(tc.tile_pool(name="persist", bufs=1))
        att = ctx.enter_context(tc.tile_pool(name="att", bufs=1))
        ps_att = ctx.enter_context(tc.tile_pool(name="ps_att", bufs=2, space="PSUM"))

        # ---- persistent SBUF ----
        q_sb = [persist.tile([128, T], BF16, tag=f"q{p}", name=f"q{p}") for p in range(2)]
        k_sb = [persist.tile([128, T], BF16, tag=f"k{p}", name=f"k{p}") for p in range(2)]
        v_sb = persist.tile([128, HPC, TB, 65], BF16, tag="v")
        y_sb = [persist.tile([128, T], BF16, tag=f"y{p}", name=f"y{p}") for p in range(2)]
        wo_sb = persist.tile([128, 2, C], BF16, tag="wo")
        mask_sb = persist.tile([128, 256], BF16, tag="mask")
        ident_sb = persist.tile([128, 128], BF16, tag="ident")

        nc.vector.memset(v_sb[:, :, :, 64], 1.0)

        # ---- BC-phase SBUF + PSUM (closes mid-kernel, LIFO on top) ----
        bc_ctx = ExitStack()
        bcp = bc_ctx.enter_context(tc.tile_pool(name="bcp", bufs=1))
        ps_bc = bc_ctx.enter_context(tc.tile_pool(name="ps_bc", bufs=2, space="PSUM"))

        xT_sb = bcp.tile([128, CK, T], BF16, tag="xT")
        wq_sb = bcp.tile([128, 2, CK, 128], BF16, tag="wq")
        wk_sb = bcp.tile([128, 2, CK, 128], BF16, tag="wk")
        wv_sb = bcp.tile([128, CK, 256], BF16, tag="wv")

        # ---- input DMAs ----
        # SP (hwdge, fast issue): critical path pieces for the first QK groups
        nc.sync.dma_start(wq_sb[:, 0], wq[0])
        nc.sync.dma_start(xT_sb[:, 0, 0:512], xT[0, :, 0:512])
        nc.sync.dma_start(xT_sb[:, 1, 0:512], xT[1, :, 0:512])
        nc.sync.dma_start(wk_sb[:, 0], wk[0])
        for ck in range(2, CK):
            nc.sync.dma_start(xT_sb[:, ck, 0:512], xT[ck, :, 0:512])
        nc.sync.dma_start(mask_sb[:], mask_d[:])
        nc.sync.dma_start(wv_sb[:], wv[:])
        for ck in range(CK):
            nc.sync.dma_start(xT_sb[:, ck, 1024:1536], xT[ck, :, 1024:1536])
        # Pool (swdge): the rest
        for ck in range(CK):
            nc.gpsimd.dma_start(xT_sb[:, ck, 512:1024], xT[ck, :, 512:1024])
        nc.gpsimd.dma_start(wq_sb[:, 1], wq[1])
        nc.gpsimd.dma_start(wk_sb[:, 1], wk[1])
        for ck in range(CK):
            nc.gpsimd.dma_start(xT_sb[:, ck, 1536:2048], xT[ck, :, 1536:2048])
        for cl in range(2):
            nc.gpsimd.dma_start(wo_sb[:, cl, :], wo[cl])
        nc.gpsimd.dma_start(ident_sb[:], ident_d[:])

        # ---- chunk emitters ----
        pb_tiles = {}    # (h, tcn) -> [pb pair tiles]
        ynat_tiles = {}  # (p, tb) -> tile

        def emit_QK(p, which, tcn):
            w_s = wq_sb if which == "q" else wk_sb
            dst = q_sb if which == "q" else k_sb
            g = ps_bc.tile([128, 512], F32, tag="bc", name="bcps")
            for ck in range(CK):
                nc.tensor.matmul(
                    g[:], lhsT=w_s[:, p, ck, :], rhs=xT_sb[:, ck, ts(tcn, 512)],
                    start=(ck == 0), stop=(ck == CK - 1),
                )
            nc.vector.tensor_copy(dst[p][:, ts(tcn, 512)], g[:])

        def emit_V(tb):
            g = ps_bc.tile([128, 512], F32, tag="bc", name="bcps")
            for ck in range(CK):
                nc.tensor.matmul(
                    g[:, 0:256], lhsT=xT_sb[:, ck, ts(tb, 128)], rhs=wv_sb[:, ck, :],
                    start=(ck == 0), stop=(ck == CK - 1),
                )
            nc.vector.tensor_copy(
                v_sb[:, :, tb, 0:64],
                g[:, 0:256].rearrange("p (h a) -> p h a", h=HPC),
            )

        def emit_S_pair(h, tcn, i, full=False):
            # `full=True` disables width-trimming: used for the first two
            # emitted pairs so the "s" PSUM ring slots start fully written
            # (later trimmed writes then expose only finite stale scores
            # to the batched exp).
            p, hl = h // 2, h % 2
            b0 = 64 * hl
            st = ps_att.tile([128, 1024], F32, tag="s", name="sps")
            pbt = att.tile([128, 1024], BF16, tag="pb", name="pb", bufs=52)
            pb_tiles.setdefault((h, tcn), []).append(pbt)
            d1 = None
            for half in range(2):
                sb = 2 * i + half
                d = sb - 4 * tcn
                off = 128 * d if (d > 0 and not full) else 0
                nc.tensor.matmul(
                    st[:, 512 * half + off: 512 * half + 512],
                    lhsT=k_sb[p][b0:b0 + 64, ts(sb, 128)],
                    rhs=q_sb[p][b0:b0 + 64, 512 * tcn + off: 512 * tcn + 512],
                    start=True, stop=True,
                )
                if half == 1:
                    d1 = d
            if d1 is not None and d1 >= 2 and not full:
                # diagonal (d2,d3) pair: only the tail regions are live
                nc.scalar.activation(pbt[:, 256:512], st[:, 256:512], AF.Exp, scale=SCALE)
                nc.scalar.activation(pbt[:, 896:1024], st[:, 896:1024], AF.Exp, scale=SCALE)
            else:
                nc.scalar.activation(pbt[:], st[:], AF.Exp, scale=SCALE)
            if d1 is not None and d1 >= 0:
                # mask the two boundary triangle blocks: cols {o, o+640}
                o = 0 if d1 == 1 else 256
                pbv = pbt[:].rearrange("p (k f) -> p k f", f=128)
                mv = mask_sb[:].rearrange("p (k f) -> p k f", f=128)
                nc.vector.tensor_mul(
                    pbv[:, o // 128: 8: 5, :], pbv[:, o // 128: 8: 5, :], mv)

        def emit_A_j(h, tcn, j, pe_transpose=None):
            tb = 4 * tcn + j
            p, hl = h // 2, h % 2
            nsb = 4 * tcn + j + 1
            yt = ps_att.tile([128, 512], F32, tag="y", name="yps")
            tiles = pb_tiles[(h, tcn)]
            for sb in range(nsb):
                pbt = tiles[sb // 2]
                o = 512 * (sb % 2) + 128 * j
                nc.tensor.matmul(
                    yt[:, 0:65], lhsT=pbt[:, o:o + 128], rhs=v_sb[:, h, sb, :],
                    start=(sb == 0), stop=(sb == nsb - 1),
                )
            rl = att.tile([128, 1], F32, tag="rl", name="rl", bufs=8)
            nc.vector.reciprocal(rl[:], yt[:, 64:65])
            if hl == 0:
                yn = att.tile([128, 128], BF16, tag="yn", name="yn", bufs=16)
                ynat_tiles[(p, tb)] = yn
            else:
                yn = ynat_tiles[(p, tb)]
            nc.vector.tensor_scalar_mul(yn[:, 64 * hl:64 * hl + 64], yt[:, 0:64], rl[:])
            if hl == 1:
                if pe_transpose is not None:
                    # latency-critical tail: PE transpose via identity + DVE
                    # evac beats the ~2.3us DMA-transpose round trip
                    trp = pe_transpose.tile([128, 1024], BF16, tag="o", name="trp")
                    nc.tensor.transpose(trp[:, 0:128], yn[:], ident_sb[:])
                    nc.vector.tensor_copy(y_sb[p][:, ts(tb, 128)], trp[:, 0:128])
                else:
                    nc.sync.dma_start_transpose(y_sb[p][:, ts(tb, 128)], yn[:])

        def emit_A(h, tcn, pe_transpose=None):
            for j in range(4):
                emit_A_j(h, tcn, j, pe_transpose=pe_transpose)

        def emit_P_tb(tb, ob, ps_o):
            for cc in range(2):
                op = ps_o.tile([128, 512], F32, tag="o", name="ops")
                for cl in range(2):
                    nc.tensor.matmul(
                        op[:], lhsT=y_sb[cl][:, ts(tb, 128)],
                        rhs=wo_sb[:, cl, ts(cc, 512)],
                        start=(cl == 0), stop=(cl == 1),
                    )
                ot = ob.tile([128, 512], BF16, tag="ot", name="ot", bufs=4)
                nc.vector.tensor_copy(ot[:], op[:])
                nc.sync.dma_start(out[tb][:, ts(cc, 512)], ot[:])

        # ---- emission schedule ----
        S = emit_S_pair
        emit_QK(0, "q", 0); emit_QK(0, "k", 0)
        S(0, 0, 0, full=True); S(0, 0, 1, full=True)
        emit_QK(0, "q", 1); emit_QK(0, "k", 1)
        S(0, 1, 0); S(0, 1, 1); S(0, 1, 2); S(0, 1, 3)
        S(1, 0, 0); S(1, 0, 1)
        emit_QK(0, "q", 2); emit_QK(0, "k", 2)
        S(1, 1, 0); S(1, 1, 1); S(1, 1, 2); S(1, 1, 3)
        emit_V(0); emit_V(1)
        emit_QK(0, "q", 3); emit_QK(0, "k", 3)
        emit_V(2); emit_V(3)
        emit_A(0, 0)
        for i in range(6):
            S(0, 2, i)
        emit_QK(1, "q", 0); emit_QK(1, "k", 0)
        emit_A(1, 0)
        for i in range(8):
            S(0, 3, i)
        emit_QK(1, "q", 1); emit_QK(1, "k", 1)
        for i in range(6):
            S(1, 2, i)
        emit_V(4); emit_V(5); emit_V(6); emit_V(7)
        emit_QK(1, "q", 2); emit_QK(1, "k", 2)
        for i in range(8):
            S(1, 3, i)
        emit_A(0, 1)
        emit_QK(1, "q", 3); emit_QK(1, "k", 3)
        S(2, 0, 0); S(2, 0, 1); S(3, 0, 0); S(3, 0, 1)
        emit_A(1, 1)
        emit_V(8); emit_V(9); emit_V(10); emit_V(11)
        for i in range(4):
            S(2, 1, i)
        for i in range(4):
            S(3, 1, i)
        emit_V(12); emit_V(13); emit_V(14); emit_V(15)

        # BC done: release its PSUM banks + SBUF, open projection pools
        bc_ctx.close()
        pr_ctx = ExitStack()
        ps_o = pr_ctx.enter_context(tc.tile_pool(name="ps_o", bufs=2, space="PSUM"))
        ob = pr_ctx.enter_context(tc.tile_pool(name="ob", bufs=1))

        def P(tcn):
            for tb in range(4 * tcn, 4 * tcn + 4):
                emit_P_tb(tb, ob, ps_o)

        emit_A(2, 0); emit_A(3, 0)
        emit_A(0, 3)
        for i in range(6):
            S(2, 2, i)
        P(0)
        emit_A(2, 1); emit_A(3, 1)
        emit_A(1, 3)
        emit_A(0, 2); emit_A(1, 2)
        # interleave the remaining scores so the late exps start early
        S(3, 2, 0); S(3, 2, 1); S(3, 2, 2)
        S(2, 3, 0); S(2, 3, 1); S(2, 3, 2); S(2, 3, 3)
        S(3, 2, 3); S(3, 2, 4); S(3, 2, 5)
        S(3, 3, 0); S(3, 3, 1)
        S(2, 3, 4); S(2, 3, 5); S(2, 3, 6); S(2, 3, 7)
        S(3, 3, 2); S(3, 3, 3)
        P(1)
        S(3, 3, 4); S(3, 3, 5)
        emit_A(2, 2); emit_A(3, 2)
        S(3, 3, 6); S(3, 3, 7)
        emit_A(2, 3)
        P(2)
        emit_A(3, 3, pe_transpose=ps_o)
        P(3)
        pr_ctx.close()

    nc.compile()
    return nc


_NC = None


def _get_nc():
    global _NC
    if _NC is None:
        _NC = build_nc()
    return _NC


def _mask_arr():
    p = np.arange(128)[:, None]
    f = np.arange(128)[None, :]
    tri = (p <= f).astype(np.float32)
    return np.ascontiguousarray(np.concatenate([tri, tri], axis=1).astype(BF))


def make_in_maps(x, w_q, w_k, w_v, w_o):
    x = np.asarray(x, dtype=np.float32)
    w_q = np.asarray(w_q, dtype=np.float32)
    w_k = np.asarray(w_k, dtype=np.float32)
    w_v = np.asarray(w_v, dtype=np.float32)
    w_o = np.asarray(w_o, dtype=np.float32)
    in_maps = []
    for c in range(NCORES):
        b, g = c // 4, c % 4
        hs = [4 * g + i for i in range(HPC)]
        xT = np.ascontiguousarray(x[b].T).reshape(CK, 128, T)
        wq_a = np.stack([
            np.concatenate([w_q[hs[2 * p]], w_q[hs[2 * p + 1]]], axis=1)
            .reshape(CK, 128, 128).transpose(1, 0, 2)
            for p in range(2)
        ])
        wk_a = np.stack([
            np.concatenate([w_k[hs[2 * p]], w_k[hs[2 * p + 1]]], axis=1)
            .reshape(CK, 128, 128).transpose(1, 0, 2)
            for p in range(2)
        ])
        wv_a = np.concatenate([w_v[h] for h in hs], axis=1).reshape(CK, 128, 256).transpose(1, 0, 2)
        wo_a = w_o[256 * g:256 * (g + 1)].reshape(2, 128, C)
        in_maps.append(dict(
            mask=_mask_arr(),
            ident=np.eye(128, dtype=BF),
            xT=np.ascontiguousarray(xT.astype(BF)),
            wq=np.ascontiguousarray(wq_a.astype(BF)),
            wk=np.ascontiguousarray(wk_a.astype(BF)),
            wv=np.ascontiguousarray(wv_a.astype(BF)),
            wo=np.ascontiguousarray(wo_a.astype(BF)),
        ))
    return in_maps


def gather_out(results):
    acc = [np.zeros((T, C), np.float64) for _ in range(B)]
    for c in range(NCORES):
        acc[c // 4] += results[c]["out"].reshape(T, C).astype(np.float64)
    return np.stack([a.astype(np.float32) for a in acc])


def run(x, w_q, w_k, w_v, w_o, trace=False, **spmd_kwargs):
    nc = _get_nc()
    in_maps = make_in_maps(x, w_q, w_k, w_v, w_o)
    res = run_bass_kernel_spmd(nc, in_maps, list(range(NCORES)), trace=trace,
                               **spmd_kwargs)
    return gather_out(res.results), res


def kernel(x, w_q, w_k, w_v, w_o):
    out, _ = run(x, w_q, w_k, w_v, w_o)
    return np.asarray(out, dtype=np.float32)


# revision 33
# speedup vs baseline: 1.4860x; 1.0110x over previous
"""Causal multi-head attention (B=2,T=2048,C=1024,H=16,Ca=64) on 8 trn2 cores.

Sharding: the 32 (batch, head) pairs are split across 8 cores - core c gets
batch b = c//4 and heads [4g, 4g+4) where g = c%4.  Each core computes its
heads' attention plus the partial output projection through its 256-row slice
of w_o; the host sums the 4 partials per batch.

v2 design (all matmul operands bf16, fp32 PSUM accumulation):
  - Q^T/K^T per head pair [128(2x64a), T]; V natural [s, a] with a ones
    column at a=64 so the A@V matmul also emits softmax row sums.
  - Scores S^T[s,t] per s-block at t-chunk (512) granularity, diagonal
    blocks width-trimmed; exp on ACT; causal triangle mask multiplied on
    the (single) boundary 128x128 block of each diagonal s-block.
  - A@V transposed: lhsT = p-block [s,128t], rhs = v [s,65] -> y natural
    [t, 64+l].  Softmax division via per-partition reciprocal+scalar-mul
    (l is a per-partition scalar in this layout).  Pair tiles [t,128c]
    are transposed back to y^T via DMA-transpose for the w_o projection.
  - Hand-interleaved emission order keeps PE fed while ACT (exp) streams.
"""

import math
import sys

import numpy as np

for _p in ("/opt/trn_rl_repo",):
    if _p not in sys.path:
        sys.path.insert(0, _p)

import ml_dtypes
import concourse.bass as bass
from concourse import bacc
import concourse.mybir as mybir
from concourse.bass import ts
from concourse.tile import TileContext
from concourse.bass_utils import run_bass_kernel_spmd
from contextlib import ExitStack

F32 = mybir.dt.float32
BF16 = mybir.dt.bfloat16
AF = mybir.ActivationFunctionType
BF = ml_dtypes.bfloat16

B, T, C = 2, 2048, 1024
H, CA = 16, 64
SCALE = 1.0 / math.sqrt(CA)
NCORES = 8
HPC = 4          # heads per core
TB = T // 128    # 16 t-blocks of 128
TC = T // 512    # 4 t-chunks of 512
CK = C // 128    # 8 c-chunks


def build_nc():
    nc = bacc.Bacc()
    xT = nc.declare_dram_parameter("xT", [CK, 128, T], BF16, isOutput=False)
    # weights pre-transposed on host so partition dim is outermost-contiguous
    wq = nc.declare_dram_parameter("wq", [2, 128, CK, 128], BF16, isOutput=False)
    wk = nc.declare_dram_parameter("wk", [2, 128, CK, 128], BF16, isOutput=False)
    wv = nc.declare_dram_parameter("wv", [128, CK, 256], BF16, isOutput=False)
    wo = nc.declare_dram_parameter("wo", [2, 128, C], BF16, isOutput=False)
    mask_d = nc.declare_dram_parameter("mask", [128, 256], BF16, isOutput=False)
    ident_d = nc.declare_dram_parameter("ident", [128, 128], BF16, isOutput=False)
    out = nc.declare_dram_parameter("out", [TB, 128, C], BF16, isOutput=True)

    with TileContext(nc) as tc, ExitStack() as ctx:
        persist = ctx.enter_context(tc.tile_pool(name="persist", bufs=1))
        att = ctx.enter_context(tc.tile_pool(name="att", bufs=1))
        ps_att = ctx.enter_context(tc.tile_pool(name="ps_att", bufs=2, space="PSUM"))

        # ---- persistent SBUF ----
        q_sb = [persist.tile([128, T], BF16, tag=f"q{p}", name=f"q{p}") for p in range(2)]
        k_sb = [persist.tile([128, T], BF16, tag=f"k{p}", name=f"k{p}") for p in range(2)]
        v_sb = persist.tile([128, HPC, TB, 65], BF16, tag="v")
        y_sb = [persist.tile([128, T], BF16, tag=f"y{p}", name=f"y{p}") for p in range(2)]
        wo_sb = persist.tile([128, 2, C], BF16, tag="wo")
        mask_sb = persist.tile([128, 256], BF16, tag="mask")
        ident_sb = persist.tile([128, 128], BF16, tag="ident")

        nc.vector.memset(v_sb[:, :, :, 64], 1.0)

        # ---- BC-phase SBUF + PSUM (closes mid-kernel, LIFO on top) ----
        bc_ctx = ExitStack()
        bcp = bc_ctx.enter_context(tc.tile_pool(name="bcp", bufs=1))
        ps_bc = bc_ctx.enter_context(tc.tile_pool(name="ps_bc", bufs=2, space="PSUM"))

        xT_sb = bcp.tile([128, CK, T], BF16, tag="xT")
        wq_sb = bcp.tile([128, 2, CK, 128], BF16, tag="wq")
        wk_sb = bcp.tile([128, 2, CK, 128], BF16, tag="wk")
        wv_sb = bcp.tile([128, CK, 256], BF16, tag="wv")

        # ---- input DMAs ----
        # SP (hwdge, fast issue): critical path pieces for the first QK groups
        nc.sync.dma_start(wq_sb[:, 0, 0:4], wq[0, :, 0:4])
        nc.sync.dma_start(wq_sb[:, 0, 4:8], wq[0, :, 4:8])
        nc.sync.dma_start(wk_sb[:, 0, 0:4], wk[0, :, 0:4])
        nc.sync.dma_start(wk_sb[:, 0, 4:8], wk[0, :, 4:8])
        for ck in range(2, CK):
            nc.sync.dma_start(xT_sb[:, ck, 0:512], xT[ck, :, 0:512])
        nc.sync.dma_start(mask_sb[:], mask_d[:])
        nc.sync.dma_start(xT_sb[:, 6, 512:1024], xT[6, :, 512:1024])
        nc.sync.dma_start(xT_sb[:, 7, 512:1024], xT[7, :, 512:1024])
        nc.sync.dma_start(wv_sb[:], wv[:])
        for ck in range(CK):
            nc.sync.dma_start(xT_sb[:, ck, 1024:1536], xT[ck, :, 1024:1536])
        # Pool (swdge): first x pieces land in parallel with SP's weights
        nc.gpsimd.dma_start(xT_sb[:, 0, 0:512], xT[0, :, 0:512])
        nc.gpsimd.dma_start(xT_sb[:, 1, 0:512], xT[1, :, 0:512])
        for ck in range(6):
            nc.gpsimd.dma_start(xT_sb[:, ck, 512:1024], xT[ck, :, 512:1024])
        nc.gpsimd.dma_start(wq_sb[:, 1], wq[1])
        nc.gpsimd.dma_start(wk_sb[:, 1], wk[1])
        for ck in range(CK):
            nc.gpsimd.dma_start(xT_sb[:, ck, 1536:2048], xT[ck, :, 1536:2048])
        for cl in range(2):
            nc.gpsimd.dma_start(wo_sb[:, cl, :], wo[cl])
        nc.gpsimd.dma_start(ident_sb[:], ident_d[:])

        # ---- chunk emitters ----
        pb_tiles = {}    # (h, tcn) -> [pb pair tiles]
        ynat_tiles = {}  # (p, tb) -> tile

        def emit_QK(p, which, tcn):
            w_s = wq_sb if which == "q" else wk_sb
            dst = q_sb if which == "q" else k_sb
            g = ps_bc.tile([128, 512], F32, tag="bc", name="bcps")
            for ck in range(CK):
                nc.tensor.matmul(
                    g[:], lhsT=w_s[:, p, ck, :], rhs=xT_sb[:, ck, ts(tcn, 512)],
                    start=(ck == 0), stop=(ck == CK - 1),
                )
            nc.vector.tensor_copy(dst[p][:, ts(tcn, 512)], g[:])

        def emit_V(tb):
            g = ps_bc.tile([128, 512], F32, tag="bc", name="bcps")
            for ck in range(CK):
                nc.tensor.matmul(
                    g[:, 0:256], lhsT=xT_sb[:, ck, ts(tb, 128)], rhs=wv_sb[:, ck, :],
                    start=(ck == 0), stop=(ck == CK - 1),
                )
            nc.vector.tensor_copy(
                v_sb[:, :, tb, 0:64],
                g[:, 0:256].rearrange("p (h a) -> p h a", h=HPC),
            )

        # per-block tile column layout inside a pb/sps pair tile: diagonal
        # blocks are stored compacted so the exp covers exactly the written
        # region (no stale PSUM is ever read).
        #   d<=0 blocks: full 512 at 512*half;  d1: block cols[128:512] at
        #   [512:896];  d2: [256:512] at [0:256];  d3: [384:512] at [256:384]
        def _score_geom(sb, tcn):
            d = sb - 4 * tcn
            half = sb % 2
            if d == 1:
                return 128, 512, 896     # q-col offset, tile start, tile end
            if d == 2:
                return 256, 0, 256
            if d == 3:
                return 384, 256, 384
            return 0, 512 * half, 512 * half + 512

        def _pb_off(sb, tcn, j):
            d = sb - 4 * tcn
            if d == 1:
                return 384 + 128 * j
            if d == 2:
                return -256 + 128 * j
            if d == 3:
                return -128 + 128 * j
            return 512 * (sb % 2) + 128 * j

        def emit_S_pair(h, tcn, i):
            p, hl = h // 2, h % 2
            b0 = 64 * hl
            st = ps_att.tile([128, 1024], F32, tag="s", name="sps")
            pbt = att.tile([128, 1024], BF16, tag="pb", name="pb", bufs=52)
            pb_tiles.setdefault((h, tcn), []).append(pbt)
            for half in range(2):
                sb = 2 * i + half
                qoff, t0, t1 = _score_geom(sb, tcn)
                nc.tensor.matmul(
                    st[:, t0:t1],
                    lhsT=k_sb[p][b0:b0 + 64, ts(sb, 128)],
                    rhs=q_sb[p][b0:b0 + 64, 512 * tcn + qoff: 512 * tcn + 512],
                    start=True, stop=True,
                )
            d1 = 2 * i + 1 - 4 * tcn
            end = 1024 if d1 < 1 else (896 if d1 == 1 else 384)
            nc.scalar.activation(pbt[:, 0:end], st[:, 0:end], AF.Exp, scale=SCALE)
            if d1 >= 1:
                # mask the two boundary triangle blocks
                step = 4 if d1 == 1 else 2
                pbv = pbt[:].rearrange("p (k f) -> p k f", f=128)
                mv = mask_sb[:].rearrange("p (k f) -> p k f", f=128)
                nc.vector.tensor_mul(
                    pbv[:, 0:step + 1:step, :], pbv[:, 0:step + 1:step, :], mv)

        def emit_A_j(h, tcn, j, pe_transpose=None):
            tb = 4 * tcn + j
            p, hl = h // 2, h % 2
            nsb = 4 * tcn + j + 1
            yt = ps_att.tile([128, 512], F32, tag="y", name="yps")
            tiles = pb_tiles[(h, tcn)]
            for sb in range(nsb):
                pbt = tiles[sb // 2]
                o = _pb_off(sb, tcn, j)
                nc.tensor.matmul(
                    yt[:, 0:65], lhsT=pbt[:, o:o + 128], rhs=v_sb[:, h, sb, :],
                    start=(sb == 0), stop=(sb == nsb - 1),
                )
            rl = att.tile([128, 1], F32, tag="rl", name="rl", bufs=8)
            nc.vector.reciprocal(rl[:], yt[:, 64:65])
            if hl == 0:
                yn = att.tile([128, 128], BF16, tag="yn", name="yn", bufs=16)
                ynat_tiles[(p, tb)] = yn
            else:
                yn = ynat_tiles[(p, tb)]
            nc.vector.tensor_scalar_mul(yn[:, 64 * hl:64 * hl + 64], yt[:, 0:64], rl[:])
            if hl == 1:
                if pe_transpose is not None:
                    # latency-critical tail: PE transpose via identity + ACT
                    # evac beats the ~2.3us DMA-transpose round trip
                    trp = pe_transpose.tile([128, 1024], BF16, tag="o", name="trp")
                    nc.tensor.transpose(trp[:, 0:128], yn[:], ident_sb[:])
                    nc.scalar.copy(y_sb[p][:, ts(tb, 128)], trp[:, 0:128])
                else:
                    nc.sync.dma_start_transpose(y_sb[p][:, ts(tb, 128)], yn[:])

        def emit_A(h, tcn, pe_transpose=None):
            for j in range(4):
                emit_A_j(h, tcn, j, pe_transpose=pe_transpose)

        def emit_P_tb(tb, ob, ps_o, evac_act=False):
            for cc in range(2):
                op = ps_o.tile([128, 512], F32, tag="o", name="ops")
                for cl in range(2):
                    nc.tensor.matmul(
                        op[:], lhsT=y_sb[cl][:, ts(tb, 128)],
                        rhs=wo_sb[:, cl, ts(cc, 512)],
                        start=(cl == 0), stop=(cl == 1),
                    )
                ot = ob.tile([128, 512], BF16, tag="ot", name="ot", bufs=4)
                if evac_act and cc == 1:
                    # tail region: split evacs DVE/ACT so neither saturates
                    nc.scalar.copy(ot[:], op[:])
                else:
                    nc.vector.tensor_copy(ot[:], op[:])
                nc.sync.dma_start(out[tb][:, ts(cc, 512)], ot[:])

        # ---- emission schedule ----
        S = emit_S_pair
        emit_QK(0, "q", 0); emit_QK(0, "k", 0)
        S(0, 0, 0); S(0, 0, 1)
        emit_QK(0, "q", 1); emit_QK(0, "k", 1)
        S(0, 1, 0); S(0, 1, 1); S(0, 1, 2); S(0, 1, 3)
        S(1, 0, 0); S(1, 0, 1)
        emit_QK(0, "q", 2); emit_QK(0, "k", 2)
        S(1, 1, 0); S(1, 1, 1); S(1, 1, 2); S(1, 1, 3)
        emit_V(0); emit_V(1)
        emit_QK(0, "q", 3); emit_QK(0, "k", 3)
        emit_V(2); emit_V(3)
        emit_A(0, 0)
        for i in range(6):
            S(0, 2, i)
        emit_QK(1, "q", 0); emit_QK(1, "k", 0)
        emit_A(1, 0)
        for i in range(8):
            S(0, 3, i)
        emit_QK(1, "q", 1); emit_QK(1, "k", 1)
        for i in range(6):
            S(1, 2, i)
        emit_V(4); emit_V(5); emit_V(6); emit_V(7)
        emit_QK(1, "q", 2); emit_QK(1, "k", 2)
        for i in range(8):
            S(1, 3, i)
        emit_A(0, 1)
        emit_QK(1, "q", 3); emit_QK(1, "k", 3)
        S(2, 0, 0); S(2, 0, 1); S(3, 0, 0); S(3, 0, 1)
        emit_A(1, 1)
        emit_V(8); emit_V(9); emit_V(10); emit_V(11)
        for i in range(4):
            S(2, 1, i)
        for i in range(4):
            S(3, 1, i)
        emit_V(12); emit_V(13); emit_V(14); emit_V(15)

        # BC done: release its PSUM banks + SBUF, open projection pools
        bc_ctx.close()
        pr_ctx = ExitStack()
        ps_o = pr_ctx.enter_context(tc.tile_pool(name="ps_o", bufs=2, space="PSUM"))
        ob = pr_ctx.enter_context(tc.tile_pool(name="ob", bufs=1))

        def P(tcn):
            for tb in range(4 * tcn, 4 * tcn + 4):
                emit_P_tb(tb, ob, ps_o)

        emit_A(2, 0); emit_A(3, 0)
        emit_A(0, 3)
        for i in range(6):
            S(2, 2, i)
        P(0)
        emit_A(2, 1); emit_A(3, 1)
        emit_A(1, 3)
        emit_A(0, 2); emit_A(1, 2)
        # interleave the remaining scores so the late exps start early
        S(3, 2, 0); S(3, 2, 1); S(3, 2, 2)
        S(2, 3, 0); S(2, 3, 1); S(2, 3, 2); S(2, 3, 3)
        S(3, 2, 3); S(3, 2, 4); S(3, 2, 5)
        S(3, 3, 0); S(3, 3, 1)
        S(2, 3, 4); S(2, 3, 5); S(2, 3, 6); S(2, 3, 7)
        S(3, 3, 2); S(3, 3, 3)
        P(1)
        S(3, 3, 4)
        emit_A_j(2, 2, 0); emit_A_j(2, 2, 1)
        S(3, 3, 5)
        emit_A_j(2, 2, 2); emit_A_j(2, 2, 3)
        S(3, 3, 6)
        emit_A_j(3, 2, 0); emit_A_j(3, 2, 1)
        S(3, 3, 7)
        emit_A_j(3, 2, 2); emit_A_j(3, 2, 3)
        emit_A_j(2, 3, 0); emit_P_tb(8, ob, ps_o, evac_act=True)
        emit_A_j(2, 3, 1); emit_P_tb(9, ob, ps_o, evac_act=True)
        emit_A_j(2, 3, 2); emit_P_tb(10, ob, ps_o, evac_act=True)
        emit_A_j(2, 3, 3); emit_P_tb(11, ob, ps_o, evac_act=True)
        emit_A_j(3, 3, 0, pe_transpose=ps_o)
        emit_A_j(3, 3, 1, pe_transpose=ps_o)
        emit_P_tb(12, ob, ps_o, evac_act=True)
        emit_A_j(3, 3, 2, pe_transpose=ps_o)
        emit_P_tb(13, ob, ps_o, evac_act=True)
        emit_A_j(3, 3, 3, pe_transpose=ps_o)
        emit_P_tb(14, ob, ps_o, evac_act=True)
        emit_P_tb(15, ob, ps_o, evac_act=True)
        pr_ctx.close()

    nc.compile()
    return nc


_NC = None


def _get_nc():
    global _NC
    if _NC is None:
        _NC = build_nc()
    return _NC


def _mask_arr():
    p = np.arange(128)[:, None]
    f = np.arange(128)[None, :]
    tri = (p <= f).astype(np.float32)
    return np.ascontiguousarray(np.concatenate([tri, tri], axis=1).astype(BF))


def make_in_maps(x, w_q, w_k, w_v, w_o):
    x = np.asarray(x, dtype=np.float32)
    w_q = np.asarray(w_q, dtype=np.float32)
    w_k = np.asarray(w_k, dtype=np.float32)
    w_v = np.asarray(w_v, dtype=np.float32)
    w_o = np.asarray(w_o, dtype=np.float32)
    in_maps = []
    for c in range(NCORES):
        b, g = c // 4, c % 4
        hs = [4 * g + i for i in range(HPC)]
        xT = np.ascontiguousarray(x[b].T).reshape(CK, 128, T)
        wq_a = np.stack([
            np.concatenate([w_q[hs[2 * p]], w_q[hs[2 * p + 1]]], axis=1)
            .reshape(CK, 128, 128).transpose(1, 0, 2)
            for p in range(2)
        ])
        wk_a = np.stack([
            np.concatenate([w_k[hs[2 * p]], w_k[hs[2 * p + 1]]], axis=1)
            .reshape(CK, 128, 128).transpose(1, 0, 2)
            for p in range(2)
        ])
        wv_a = np.concatenate([w_v[h] for h in hs], axis=1).reshape(CK, 128, 256).transpose(1, 0, 2)
        wo_a = w_o[256 * g:256 * (g + 1)].reshape(2, 128, C)
        in_maps.append(dict(
            mask=_mask_arr(),
            ident=np.eye(128, dtype=BF),
            xT=np.ascontiguousarray(xT.astype(BF)),
            wq=np.ascontiguousarray(wq_a.astype(BF)),
            wk=np.ascontiguousarray(wk_a.astype(BF)),
            wv=np.ascontiguousarray(wv_a.astype(BF)),
            wo=np.ascontiguousarray(wo_a.astype(BF)),
        ))
    return in_maps


def gather_out(results):
    acc = [np.zeros((T, C), np.float64) for _ in range(B)]
    for c in range(NCORES):
        acc[c // 4] += results[c]["out"].reshape(T, C).astype(np.float64)
    return np.stack([a.astype(np.float32) for a in acc])


def run(x, w_q, w_k, w_v, w_o, trace=False, **spmd_kwargs):
    nc = _get_nc()
    in_maps = make_in_maps(x, w_q, w_k, w_v, w_o)
    res = run_bass_kernel_spmd(nc, in_maps, list(range(NCORES)), trace=trace,
                               **spmd_kwargs)
    return gather_out(res.results), res


def kernel(x, w_q, w_k, w_v, w_o):
    out, _ = run(x, w_q, w_k, w_v, w_o)
    return np.asarray(out, dtype=np.float32)


# revision 46
# speedup vs baseline: 1.5191x; 1.0223x over previous
"""Causal multi-head attention (B=2,T=2048,C=1024,H=16,Ca=64) on 8 trn2 cores.

Sharding: the 32 (batch, head) pairs are split across 8 cores - core c gets
batch b = c//4 and heads [4g, 4g+4) where g = c%4.  Each core computes its
heads' attention plus the partial output projection through its 256-row slice
of w_o; the host sums the 4 partials per batch.

v2 design (all matmul operands bf16, fp32 PSUM accumulation):
  - Q^T/K^T per head pair [128(2x64a), T]; V natural [s, a] with a ones
    column at a=64 so the A@V matmul also emits softmax row sums.
  - Scores S^T[s,t] per s-block at t-chunk (512) granularity, diagonal
    blocks width-trimmed; exp on ACT; causal triangle mask multiplied on
    the (single) boundary 128x128 block of each diagonal s-block.
  - A@V transposed: lhsT = p-block [s,128t], rhs = v [s,65] -> y natural
    [t, 64+l].  Softmax division via per-partition reciprocal+scalar-mul
    (l is a per-partition scalar in this layout).  Pair tiles [t,128c]
    are transposed back to y^T via DMA-transpose for the w_o projection.
  - Hand-interleaved emission order keeps PE fed while ACT (exp) streams.
"""

import math
import sys

import numpy as np

for _p in ("/opt/trn_rl_repo",):
    if _p not in sys.path:
        sys.path.insert(0, _p)

import ml_dtypes
import concourse.bass as bass
from concourse import bacc
import concourse.mybir as mybir
from concourse.bass import ts
from concourse.tile import TileContext
from concourse.bass_utils import run_bass_kernel_spmd
from contextlib import ExitStack

F32 = mybir.dt.float32
BF16 = mybir.dt.bfloat16
AF = mybir.ActivationFunctionType
BF = ml_dtypes.bfloat16

B, T, C = 2, 2048, 1024
H, CA = 16, 64
SCALE = 1.0 / math.sqrt(CA)
NCORES = 8
HPC = 4          # heads per core
TB = T // 128    # 16 t-blocks of 128
TC = T // 512    # 4 t-chunks of 512
CK = C // 128    # 8 c-chunks


def build_nc():
    nc = bacc.Bacc()
    xT = nc.declare_dram_parameter("xT", [CK, 128, T], BF16, isOutput=False)
    # weights pre-transposed on host so partition dim is outermost-contiguous
    wq = nc.declare_dram_parameter("wq", [2, 128, CK, 128], BF16, isOutput=False)
    wk = nc.declare_dram_parameter("wk", [2, 128, CK, 128], BF16, isOutput=False)
    wv = nc.declare_dram_parameter("wv", [128, CK, 256], BF16, isOutput=False)
    wo = nc.declare_dram_parameter("wo", [2, 128, C], BF16, isOutput=False)
    mask_d = nc.declare_dram_parameter("mask", [128, 256], BF16, isOutput=False)
    ident_d = nc.declare_dram_parameter("ident", [128, 128], BF16, isOutput=False)
    out = nc.declare_dram_parameter("out", [TB, 128, C], BF16, isOutput=True)

    with TileContext(nc) as tc, ExitStack() as ctx:
        persist = ctx.enter_context(tc.tile_pool(name="persist", bufs=1))
        att = ctx.enter_context(tc.tile_pool(name="att", bufs=1))
        ps_att = ctx.enter_context(tc.tile_pool(name="ps_att", bufs=2, space="PSUM"))

        # ---- persistent SBUF ----
        q_sb = [persist.tile([128, T], BF16, tag=f"q{p}", name=f"q{p}") for p in range(2)]
        k_sb = [persist.tile([128, T], BF16, tag=f"k{p}", name=f"k{p}") for p in range(2)]
        v_sb = persist.tile([128, HPC, TB, 65], BF16, tag="v")
        y_sb = [persist.tile([128, T], BF16, tag=f"y{p}", name=f"y{p}") for p in range(2)]
        wo_sb = persist.tile([128, 2, C], BF16, tag="wo")
        mask_sb = persist.tile([128, 256], BF16, tag="mask")
        ident_sb = persist.tile([128, 128], BF16, tag="ident")

        nc.vector.memset(v_sb[:, :, :, 64], 1.0)

        # ---- BC-phase SBUF + PSUM (closes mid-kernel, LIFO on top) ----
        bc_ctx = ExitStack()
        bcp = bc_ctx.enter_context(tc.tile_pool(name="bcp", bufs=1))
        ps_bc = bc_ctx.enter_context(tc.tile_pool(name="ps_bc", bufs=2, space="PSUM"))

        xT_sb = bcp.tile([128, CK, T], BF16, tag="xT")
        wq_sb = bcp.tile([128, 2, CK, 128], BF16, tag="wq")
        wk_sb = bcp.tile([128, 2, CK, 128], BF16, tag="wk")
        wv_sb = bcp.tile([128, CK, 256], BF16, tag="wv")

        # ---- input DMAs ----
        # SP (hwdge, fast issue): critical path pieces for the first QK groups
        nc.sync.dma_start(wq_sb[:, 0, 0:4], wq[0, :, 0:4])
        nc.sync.dma_start(wq_sb[:, 0, 4:8], wq[0, :, 4:8])
        nc.sync.dma_start(wk_sb[:, 0, 0:4], wk[0, :, 0:4])
        nc.sync.dma_start(wk_sb[:, 0, 4:8], wk[0, :, 4:8])
        for ck in range(2, CK):
            nc.sync.dma_start(xT_sb[:, ck, 0:512], xT[ck, :, 0:512])
        nc.sync.dma_start(mask_sb[:], mask_d[:])
        nc.sync.dma_start(xT_sb[:, 6, 512:1024], xT[6, :, 512:1024])
        nc.sync.dma_start(xT_sb[:, 7, 512:1024], xT[7, :, 512:1024])
        nc.sync.dma_start(wv_sb[:], wv[:])
        for ck in range(CK):
            nc.sync.dma_start(xT_sb[:, ck, 1024:1536], xT[ck, :, 1024:1536])
        # Pool (swdge): first x pieces land in parallel with SP's weights
        nc.gpsimd.dma_start(xT_sb[:, 0, 0:512], xT[0, :, 0:512])
        nc.gpsimd.dma_start(xT_sb[:, 1, 0:512], xT[1, :, 0:512])
        for ck in range(6):
            nc.gpsimd.dma_start(xT_sb[:, ck, 512:1024], xT[ck, :, 512:1024])
        nc.gpsimd.dma_start(wq_sb[:, 1], wq[1])
        nc.gpsimd.dma_start(wk_sb[:, 1], wk[1])
        for ck in range(CK):
            nc.gpsimd.dma_start(xT_sb[:, ck, 1536:2048], xT[ck, :, 1536:2048])
        for cl in range(2):
            nc.gpsimd.dma_start(wo_sb[:, cl, :], wo[cl])
        nc.gpsimd.dma_start(ident_sb[:], ident_d[:])

        # ---- chunk emitters ----
        pb_tiles = {}    # (h, tcn) -> [pb pair tiles]
        ynat_tiles = {}  # (p, tb) -> tile

        def emit_QK(p, which, tcn):
            w_s = wq_sb if which == "q" else wk_sb
            dst = q_sb if which == "q" else k_sb
            g = ps_bc.tile([128, 512], F32, tag="bc", name="bcps")
            for ck in range(CK):
                nc.tensor.matmul(
                    g[:], lhsT=w_s[:, p, ck, :], rhs=xT_sb[:, ck, ts(tcn, 512)],
                    start=(ck == 0), stop=(ck == CK - 1),
                )
            nc.vector.tensor_copy(dst[p][:, ts(tcn, 512)], g[:])

        def emit_V(tb):
            g = ps_bc.tile([128, 512], F32, tag="bc", name="bcps")
            for ck in range(CK):
                nc.tensor.matmul(
                    g[:, 0:256], lhsT=xT_sb[:, ck, ts(tb, 128)], rhs=wv_sb[:, ck, :],
                    start=(ck == 0), stop=(ck == CK - 1),
                )
            nc.vector.tensor_copy(
                v_sb[:, :, tb, 0:64],
                g[:, 0:256].rearrange("p (h a) -> p h a", h=HPC),
            )

        # per-block tile column layout inside a pb/sps pair tile: diagonal
        # blocks are stored compacted so the exp covers exactly the written
        # region (no stale PSUM is ever read).
        #   d<=0 blocks: full 512 at 512*half;  d1: block cols[128:512] at
        #   [512:896];  d2: [256:512] at [0:256];  d3: [384:512] at [256:384]
        def _score_geom(sb, tcn):
            d = sb - 4 * tcn
            half = sb % 2
            if d == 1:
                return 128, 512, 896     # q-col offset, tile start, tile end
            if d == 2:
                return 256, 0, 256
            if d == 3:
                return 384, 256, 384
            return 0, 512 * half, 512 * half + 512

        def _pb_off(sb, tcn, j):
            d = sb - 4 * tcn
            if d == 1:
                return 384 + 128 * j
            if d == 2:
                return -256 + 128 * j
            if d == 3:
                return -128 + 128 * j
            return 512 * (sb % 2) + 128 * j

        def emit_S_pair(h, tcn, i):
            p, hl = h // 2, h % 2
            b0 = 64 * hl
            st = ps_att.tile([128, 1024], F32, tag="s", name="sps")
            pbt = att.tile([128, 1024], BF16, tag="pb", name="pb", bufs=52)
            pb_tiles.setdefault((h, tcn), []).append(pbt)
            for half in range(2):
                sb = 2 * i + half
                qoff, t0, t1 = _score_geom(sb, tcn)
                nc.tensor.matmul(
                    st[:, t0:t1],
                    lhsT=k_sb[p][b0:b0 + 64, ts(sb, 128)],
                    rhs=q_sb[p][b0:b0 + 64, 512 * tcn + qoff: 512 * tcn + 512],
                    start=True, stop=True,
                )
            d1 = 2 * i + 1 - 4 * tcn
            end = 1024 if d1 < 1 else (896 if d1 == 1 else 384)
            nc.scalar.activation(pbt[:, 0:end], st[:, 0:end], AF.Exp, scale=SCALE)
            if d1 >= 1:
                # mask the two boundary triangle blocks
                step = 4 if d1 == 1 else 2
                pbv = pbt[:].rearrange("p (k f) -> p k f", f=128)
                mv = mask_sb[:].rearrange("p (k f) -> p k f", f=128)
                nc.vector.tensor_mul(
                    pbv[:, 0:step + 1:step, :], pbv[:, 0:step + 1:step, :], mv)

        def emit_A_j(h, tcn, j, pe_transpose=None):
            tb = 4 * tcn + j
            p, hl = h // 2, h % 2
            nsb = 4 * tcn + j + 1
            yt = ps_att.tile([128, 512], F32, tag="y", name="yps")
            tiles = pb_tiles[(h, tcn)]
            for sb in range(nsb):
                pbt = tiles[sb // 2]
                o = _pb_off(sb, tcn, j)
                nc.tensor.matmul(
                    yt[:, 0:65], lhsT=pbt[:, o:o + 128], rhs=v_sb[:, h, sb, :],
                    start=(sb == 0), stop=(sb == nsb - 1),
                )
            rl = att.tile([128, 1], F32, tag="rl", name="rl", bufs=8)
            nc.vector.reciprocal(rl[:], yt[:, 64:65])
            if hl == 0:
                yn = att.tile([128, 128], BF16, tag="yn", name="yn", bufs=16)
                ynat_tiles[(p, tb)] = yn
            else:
                yn = ynat_tiles[(p, tb)]
            nc.vector.tensor_scalar_mul(yn[:, 64 * hl:64 * hl + 64], yt[:, 0:64], rl[:])
            if hl == 1:
                if pe_transpose is not None:
                    # latency-critical tail: PE transpose via identity + ACT
                    # evac beats the ~2.3us DMA-transpose round trip
                    trp = pe_transpose.tile([128, 1024], BF16, tag="o", name="trp")
                    nc.tensor.transpose(trp[:, 0:128], yn[:], ident_sb[:])
                    nc.vector.tensor_copy(y_sb[p][:, ts(tb, 128)], trp[:, 0:128])
                else:
                    nc.sync.dma_start_transpose(y_sb[p][:, ts(tb, 128)], yn[:])

        def emit_A(h, tcn, pe_transpose=None):
            for j in range(4):
                emit_A_j(h, tcn, j, pe_transpose=pe_transpose)

        def emit_P_tb(tb, ob, ps_o, evac_act=False, alt_y=False, split_dma=False):
            ot = ob.tile([128, 1024], BF16, tag="ot", name="ot", bufs=3)
            for cc in range(2):
                if alt_y and cc == 1:
                    # borrow the (idle) AV psum ring to double effective depth
                    op = ps_att.tile([128, 512], F32, tag="y", name="opsy")
                else:
                    op = ps_o.tile([128, 512], F32, tag="o", name="ops")
                for cl in range(2):
                    nc.tensor.matmul(
                        op[:], lhsT=y_sb[cl][:, ts(tb, 128)],
                        rhs=wo_sb[:, cl, ts(cc, 512)],
                        start=(cl == 0), stop=(cl == 1),
                    )
                if evac_act and cc == 1:
                    # tail region: split evacs DVE/ACT so neither saturates
                    nc.scalar.copy(ot[:, ts(cc, 512)], op[:])
                else:
                    nc.vector.tensor_copy(ot[:, ts(cc, 512)], op[:])
                if split_dma:
                    nc.sync.dma_start(out[tb][:, ts(cc, 512)], ot[:, ts(cc, 512)])
            if not split_dma:
                nc.sync.dma_start(out[tb], ot[:])

        # ---- emission schedule ----
        S = emit_S_pair
        emit_QK(0, "q", 0); emit_QK(0, "k", 0)
        S(0, 0, 0); S(0, 0, 1)
        emit_QK(0, "q", 1); emit_QK(0, "k", 1)
        S(0, 1, 0); S(0, 1, 1); S(0, 1, 2); S(0, 1, 3)
        S(1, 0, 0); S(1, 0, 1)
        emit_QK(0, "q", 2); emit_QK(0, "k", 2)
        S(1, 1, 0); S(1, 1, 1); S(1, 1, 2); S(1, 1, 3)
        emit_V(0); emit_V(1)
        emit_QK(0, "q", 3); emit_QK(0, "k", 3)
        emit_V(2); emit_V(3)
        emit_A(0, 0)
        for i in range(6):
            S(0, 2, i)
        emit_QK(1, "q", 0); emit_QK(1, "k", 0)
        emit_A(1, 0)
        for i in range(8):
            S(0, 3, i)
        emit_QK(1, "q", 1); emit_QK(1, "k", 1)
        for i in range(6):
            S(1, 2, i)
        emit_V(4); emit_V(5); emit_V(6); emit_V(7)
        emit_QK(1, "q", 2); emit_QK(1, "k", 2)
        for i in range(8):
            S(1, 3, i)
        emit_A(0, 1)
        emit_QK(1, "q", 3); emit_QK(1, "k", 3)
        S(2, 0, 0); S(2, 0, 1); S(3, 0, 0); S(3, 0, 1)
        emit_A(1, 1)
        emit_V(8); emit_V(9); emit_V(10); emit_V(11)
        for i in range(4):
            S(2, 1, i)
        for i in range(4):
            S(3, 1, i)
        emit_V(12); emit_V(13); emit_V(14); emit_V(15)

        # BC done: release its PSUM banks + SBUF, open projection pools
        bc_ctx.close()
        pr_ctx = ExitStack()
        ps_o = pr_ctx.enter_context(tc.tile_pool(name="ps_o", bufs=2, space="PSUM"))
        ob = pr_ctx.enter_context(tc.tile_pool(name="ob", bufs=1))

        def P(tcn):
            for tb in range(4 * tcn, 4 * tcn + 4):
                emit_P_tb(tb, ob, ps_o)

        emit_A_j(2, 0, 0); emit_A_j(2, 0, 1); S(2, 2, 0)
        emit_A_j(2, 0, 2); emit_A_j(2, 0, 3); S(2, 2, 1)
        emit_A_j(3, 0, 0); emit_A_j(3, 0, 1); S(2, 2, 2)
        emit_A_j(3, 0, 2); emit_A_j(3, 0, 3); S(2, 2, 3)
        emit_A_j(0, 3, 0); S(2, 2, 4); emit_A_j(0, 3, 1); S(2, 2, 5)
        emit_A_j(0, 3, 2); emit_A_j(0, 3, 3)
        P(0)
        emit_A_j(2, 1, 0); emit_A_j(2, 1, 1); S(3, 2, 0)
        emit_A_j(2, 1, 2); emit_A_j(2, 1, 3); S(3, 2, 1)
        emit_A_j(3, 1, 0); emit_A_j(3, 1, 1); S(3, 2, 2)
        emit_A_j(3, 1, 2); emit_A_j(3, 1, 3); S(3, 2, 3)
        emit_A_j(1, 3, 0); S(3, 2, 4); emit_A_j(1, 3, 1); S(3, 2, 5)
        emit_A_j(1, 3, 2); emit_A_j(1, 3, 3)
        emit_A_j(0, 2, 0); emit_A_j(0, 2, 1); emit_A_j(0, 2, 2); emit_A_j(0, 2, 3)
        emit_A_j(1, 2, 0); emit_A_j(1, 2, 1); S(2, 3, 0)
        emit_A_j(1, 2, 2); emit_A_j(1, 2, 3); S(2, 3, 1)
        S(2, 3, 2); S(2, 3, 3)
        S(3, 3, 0); S(3, 3, 1)
        S(2, 3, 4); S(2, 3, 5); S(2, 3, 6); S(2, 3, 7)
        S(3, 3, 2); S(3, 3, 3)
        P(1)
        S(3, 3, 4)
        emit_A_j(2, 2, 0); emit_A_j(2, 2, 1)
        S(3, 3, 5)
        emit_A_j(2, 2, 2); emit_A_j(2, 2, 3)
        S(3, 3, 6)
        emit_A_j(3, 2, 0); emit_A_j(3, 2, 1)
        S(3, 3, 7)
        emit_A_j(3, 2, 2); emit_A_j(3, 2, 3)
        emit_A_j(2, 3, 0); emit_P_tb(8, ob, ps_o, evac_act=True)
        emit_A_j(2, 3, 1); emit_P_tb(9, ob, ps_o, evac_act=True)
        emit_A_j(2, 3, 2); emit_P_tb(10, ob, ps_o, evac_act=True)
        emit_A_j(2, 3, 3); emit_P_tb(11, ob, ps_o, evac_act=True)
        emit_A_j(3, 3, 0, pe_transpose=ps_o)
        emit_A_j(3, 3, 1, pe_transpose=ps_o)
        emit_A_j(3, 3, 2, pe_transpose=ps_o)
        emit_A_j(3, 3, 3, pe_transpose=ps_o)
        emit_P_tb(12, ob, ps_o, evac_act=True, alt_y=True)
        emit_P_tb(13, ob, ps_o, evac_act=True, alt_y=True)
        emit_P_tb(14, ob, ps_o, evac_act=True, alt_y=True)
        emit_P_tb(15, ob, ps_o, evac_act=True, alt_y=True, split_dma=True)
        pr_ctx.close()

    nc.compile()
    return nc


_NC = None


def _get_nc():
    global _NC
    if _NC is None:
        _NC = build_nc()
    return _NC


def _mask_arr():
    p = np.arange(128)[:, None]
    f = np.arange(128)[None, :]
    tri = (p <= f).astype(np.float32)
    return np.ascontiguousarray(np.concatenate([tri, tri], axis=1).astype(BF))


def make_in_maps(x, w_q, w_k, w_v, w_o):
    x = np.asarray(x, dtype=np.float32)
    w_q = np.asarray(w_q, dtype=np.float32)
    w_k = np.asarray(w_k, dtype=np.float32)
    w_v = np.asarray(w_v, dtype=np.float32)
    w_o = np.asarray(w_o, dtype=np.float32)
    in_maps = []
    for c in range(NCORES):
        b, g = c // 4, c % 4
        hs = [4 * g + i for i in range(HPC)]
        xT = np.ascontiguousarray(x[b].T).reshape(CK, 128, T)
        wq_a = np.stack([
            np.concatenate([w_q[hs[2 * p]], w_q[hs[2 * p + 1]]], axis=1)
            .reshape(CK, 128, 128).transpose(1, 0, 2)
            for p in range(2)
        ])
        wk_a = np.stack([
            np.concatenate([w_k[hs[2 * p]], w_k[hs[2 * p + 1]]], axis=1)
            .reshape(CK, 128, 128).transpose(1, 0, 2)
            for p in range(2)
        ])
        wv_a = np.concatenate([w_v[h] for h in hs], axis=1).reshape(CK, 128, 256).transpose(1, 0, 2)
        wo_a = w_o[256 * g:256 * (g + 1)].reshape(2, 128, C)
        in_maps.append(dict(
            mask=_mask_arr(),
            ident=np.eye(128, dtype=BF),
            xT=np.ascontiguousarray(xT.astype(BF)),
            wq=np.ascontiguousarray(wq_a.astype(BF)),
            wk=np.ascontiguousarray(wk_a.astype(BF)),
            wv=np.ascontiguousarray(wv_a.astype(BF)),
            wo=np.ascontiguousarray(wo_a.astype(BF)),
        ))
    return in_maps


def gather_out(results):
    acc = [np.zeros((T, C), np.float64) for _ in range(B)]
    for c in range(NCORES):
        acc[c // 4] += results[c]["out"].reshape(T, C).astype(np.float64)
    return np.stack([a.astype(np.float32) for a in acc])


def run(x, w_q, w_k, w_v, w_o, trace=False, **spmd_kwargs):
    nc = _get_nc()
    in_maps = make_in_maps(x, w_q, w_k, w_v, w_o)
    res = run_bass_kernel_spmd(nc, in_maps, list(range(NCORES)), trace=trace,
                               **spmd_kwargs)
    return gather_out(res.results), res


def kernel(x, w_q, w_k, w_v, w_o):
    out, _ = run(x, w_q, w_k, w_v, w_o)
    return np.asarray(out, dtype=np.float32)


# revision 54
# speedup vs baseline: 1.5192x; 1.0001x over previous
"""Causal multi-head attention (B=2,T=2048,C=1024,H=16,Ca=64) on 8 trn2 cores.

Sharding: the 32 (batch, head) pairs are split across 8 cores - core c gets
batch b = c//4 and heads [4g, 4g+4) where g = c%4.  Each core computes its
heads' attention plus the partial output projection through its 256-row slice
of w_o; the host sums the 4 partials per batch.

v2 design (all matmul operands bf16, fp32 PSUM accumulation):
  - Q^T/K^T per head pair [128(2x64a), T]; V natural [s, a] with a ones
    column at a=64 so the A@V matmul also emits softmax row sums.
  - Scores S^T[s,t] per s-block at t-chunk (512) granularity, diagonal
    blocks width-trimmed; exp on ACT; causal triangle mask multiplied on
    the (single) boundary 128x128 block of each diagonal s-block.
  - A@V transposed: lhsT = p-block [s,128t], rhs = v [s,65] -> y natural
    [t, 64+l].  Softmax division via per-partition reciprocal+scalar-mul
    (l is a per-partition scalar in this layout).  Pair tiles [t,128c]
    are transposed back to y^T via DMA-transpose for the w_o projection.
  - Hand-interleaved emission order keeps PE fed while ACT (exp) streams.
"""

import math
import sys

import numpy as np

for _p in ("/opt/trn_rl_repo",):
    if _p not in sys.path:
        sys.path.insert(0, _p)

import ml_dtypes
import concourse.bass as bass
from concourse import bacc
import concourse.mybir as mybir
from concourse.bass import ts
from concourse.tile import TileContext
from concourse.bass_utils import run_bass_kernel_spmd
from contextlib import ExitStack

F32 = mybir.dt.float32
BF16 = mybir.dt.bfloat16
AF = mybir.ActivationFunctionType
BF = ml_dtypes.bfloat16

B, T, C = 2, 2048, 1024
H, CA = 16, 64
SCALE = 1.0 / math.sqrt(CA)
NCORES = 8
HPC = 4          # heads per core
TB = T // 128    # 16 t-blocks of 128
TC = T // 512    # 4 t-chunks of 512
CK = C // 128    # 8 c-chunks


def build_nc():
    nc = bacc.Bacc()
    xT = nc.declare_dram_parameter("xT", [CK, 128, T], BF16, isOutput=False)
    # weights pre-transposed on host so partition dim is outermost-contiguous
    wq = nc.declare_dram_parameter("wq", [2, 128, CK, 128], BF16, isOutput=False)
    wk = nc.declare_dram_parameter("wk", [2, 128, CK, 128], BF16, isOutput=False)
    wv = nc.declare_dram_parameter("wv", [128, CK, 256], BF16, isOutput=False)
    wo = nc.declare_dram_parameter("wo", [2, 128, C], BF16, isOutput=False)
    mask_d = nc.declare_dram_parameter("mask", [128, 256], BF16, isOutput=False)
    ident_d = nc.declare_dram_parameter("ident", [128, 128], BF16, isOutput=False)
    out = nc.declare_dram_parameter("out", [TB, 128, C], BF16, isOutput=True)

    with TileContext(nc) as tc, ExitStack() as ctx:
        persist = ctx.enter_context(tc.tile_pool(name="persist", bufs=1))
        att = ctx.enter_context(tc.tile_pool(name="att", bufs=1))
        ps_att = ctx.enter_context(tc.tile_pool(name="ps_att", bufs=2, space="PSUM"))

        # ---- persistent SBUF ----
        q_sb = [persist.tile([128, T], BF16, tag=f"q{p}", name=f"q{p}") for p in range(2)]
        k_sb = [persist.tile([128, T], BF16, tag=f"k{p}", name=f"k{p}") for p in range(2)]
        v_sb = persist.tile([128, HPC, TB, 65], BF16, tag="v")
        y_sb = [persist.tile([128, T], BF16, tag=f"y{p}", name=f"y{p}") for p in range(2)]
        wo_sb = persist.tile([128, 2, C], BF16, tag="wo")
        mask_sb = persist.tile([128, 256], BF16, tag="mask")
        ident_sb = persist.tile([128, 128], BF16, tag="ident")

        nc.vector.memset(v_sb[:, :, :, 64], 1.0)

        # ---- BC-phase SBUF + PSUM (closes mid-kernel, LIFO on top) ----
        bc_ctx = ExitStack()
        bcp = bc_ctx.enter_context(tc.tile_pool(name="bcp", bufs=1))
        ps_bc = bc_ctx.enter_context(tc.tile_pool(name="ps_bc", bufs=2, space="PSUM"))

        xT_sb = bcp.tile([128, CK, T], BF16, tag="xT")
        wq_sb = bcp.tile([128, 2, CK, 128], BF16, tag="wq")
        wk_sb = bcp.tile([128, 2, CK, 128], BF16, tag="wk")
        wv_sb = bcp.tile([128, CK, 256], BF16, tag="wv")

        # ---- input DMAs ----
        # SP (hwdge, fast issue): critical path pieces for the first QK groups
        nc.sync.dma_start(wq_sb[:, 0, 0:4], wq[0, :, 0:4])
        nc.sync.dma_start(wq_sb[:, 0, 4:8], wq[0, :, 4:8])
        nc.sync.dma_start(wk_sb[:, 0, 0:4], wk[0, :, 0:4])
        nc.sync.dma_start(wk_sb[:, 0, 4:8], wk[0, :, 4:8])
        for ck in range(2, CK):
            nc.sync.dma_start(xT_sb[:, ck, 0:512], xT[ck, :, 0:512])
        nc.sync.dma_start(xT_sb[:, 6, 512:1024], xT[6, :, 512:1024])
        nc.sync.dma_start(mask_sb[:], mask_d[:])
        nc.sync.dma_start(xT_sb[:, 7, 512:1024], xT[7, :, 512:1024])
        nc.sync.dma_start(wv_sb[:], wv[:])
        for ck in range(CK):
            nc.sync.dma_start(xT_sb[:, ck, 1024:1536], xT[ck, :, 1024:1536])
        # Pool (swdge): first x pieces land in parallel with SP's weights
        nc.gpsimd.dma_start(xT_sb[:, 0, 0:512], xT[0, :, 0:512])
        nc.gpsimd.dma_start(xT_sb[:, 1, 0:512], xT[1, :, 0:512])
        for ck in range(6):
            nc.gpsimd.dma_start(xT_sb[:, ck, 512:1024], xT[ck, :, 512:1024])
        nc.gpsimd.dma_start(wq_sb[:, 1], wq[1])
        nc.gpsimd.dma_start(wk_sb[:, 1], wk[1])
        for ck in range(CK):
            nc.gpsimd.dma_start(xT_sb[:, ck, 1536:2048], xT[ck, :, 1536:2048])
        for cl in range(2):
            nc.gpsimd.dma_start(wo_sb[:, cl, :], wo[cl])
        nc.gpsimd.dma_start(ident_sb[:], ident_d[:])

        # ---- chunk emitters ----
        pb_tiles = {}    # (h, tcn) -> [pb pair tiles]
        ynat_tiles = {}  # (p, tb) -> tile

        def emit_QK(p, which, tcn):
            w_s = wq_sb if which == "q" else wk_sb
            dst = q_sb if which == "q" else k_sb
            g = ps_bc.tile([128, 512], F32, tag="bc", name="bcps")
            for ck in range(CK):
                nc.tensor.matmul(
                    g[:], lhsT=w_s[:, p, ck, :], rhs=xT_sb[:, ck, ts(tcn, 512)],
                    start=(ck == 0), stop=(ck == CK - 1),
                )
            nc.vector.tensor_copy(dst[p][:, ts(tcn, 512)], g[:])

        def emit_QK2(p, tcn):
            # q and k interleaved per c-chunk: matches the startup xT DMA
            # arrival rate (each chunk is consumed twice on arrival)
            gq = ps_bc.tile([128, 512], F32, tag="bc", name="bcps")
            gk = ps_bc.tile([128, 512], F32, tag="bc", name="bcps")
            for ck in range(CK):
                for w_s, g in ((wq_sb, gq), (wk_sb, gk)):
                    nc.tensor.matmul(
                        g[:], lhsT=w_s[:, p, ck, :], rhs=xT_sb[:, ck, ts(tcn, 512)],
                        start=(ck == 0), stop=(ck == CK - 1),
                    )
            nc.vector.tensor_copy(q_sb[p][:, ts(tcn, 512)], gq[:])
            nc.vector.tensor_copy(k_sb[p][:, ts(tcn, 512)], gk[:])

        def emit_V(tb):
            g = ps_bc.tile([128, 512], F32, tag="bc", name="bcps")
            for ck in range(CK):
                nc.tensor.matmul(
                    g[:, 0:256], lhsT=xT_sb[:, ck, ts(tb, 128)], rhs=wv_sb[:, ck, :],
                    start=(ck == 0), stop=(ck == CK - 1),
                )
            nc.vector.tensor_copy(
                v_sb[:, :, tb, 0:64],
                g[:, 0:256].rearrange("p (h a) -> p h a", h=HPC),
            )

        # per-block tile column layout inside a pb/sps pair tile: diagonal
        # blocks are stored compacted so the exp covers exactly the written
        # region (no stale PSUM is ever read).
        #   d<=0 blocks: full 512 at 512*half;  d1: block cols[128:512] at
        #   [512:896];  d2: [256:512] at [0:256];  d3: [384:512] at [256:384]
        def _score_geom(sb, tcn):
            d = sb - 4 * tcn
            half = sb % 2
            if d == 1:
                return 128, 512, 896     # q-col offset, tile start, tile end
            if d == 2:
                return 256, 0, 256
            if d == 3:
                return 384, 256, 384
            return 0, 512 * half, 512 * half + 512

        def _pb_off(sb, tcn, j):
            d = sb - 4 * tcn
            if d == 1:
                return 384 + 128 * j
            if d == 2:
                return -256 + 128 * j
            if d == 3:
                return -128 + 128 * j
            return 512 * (sb % 2) + 128 * j

        def emit_S_pair(h, tcn, i):
            p, hl = h // 2, h % 2
            b0 = 64 * hl
            st = ps_att.tile([128, 1024], F32, tag="s", name="sps")
            pbt = att.tile([128, 1024], BF16, tag="pb", name="pb", bufs=52)
            pb_tiles.setdefault((h, tcn), []).append(pbt)
            for half in range(2):
                sb = 2 * i + half
                qoff, t0, t1 = _score_geom(sb, tcn)
                nc.tensor.matmul(
                    st[:, t0:t1],
                    lhsT=k_sb[p][b0:b0 + 64, ts(sb, 128)],
                    rhs=q_sb[p][b0:b0 + 64, 512 * tcn + qoff: 512 * tcn + 512],
                    start=True, stop=True,
                )
            d1 = 2 * i + 1 - 4 * tcn
            end = 1024 if d1 < 1 else (896 if d1 == 1 else 384)
            nc.scalar.activation(pbt[:, 0:end], st[:, 0:end], AF.Exp, scale=SCALE)
            if d1 >= 1:
                # mask the two boundary triangle blocks
                step = 4 if d1 == 1 else 2
                pbv = pbt[:].rearrange("p (k f) -> p k f", f=128)
                mv = mask_sb[:].rearrange("p (k f) -> p k f", f=128)
                nc.gpsimd.tensor_mul(
                    pbv[:, 0:step + 1:step, :], pbv[:, 0:step + 1:step, :], mv)

        def emit_A_j(h, tcn, j, pe_transpose=None):
            tb = 4 * tcn + j
            p, hl = h // 2, h % 2
            nsb = 4 * tcn + j + 1
            yt = ps_att.tile([128, 512], F32, tag="y", name="yps")
            tiles = pb_tiles[(h, tcn)]
            for sb in range(nsb):
                pbt = tiles[sb // 2]
                o = _pb_off(sb, tcn, j)
                nc.tensor.matmul(
                    yt[:, 0:65], lhsT=pbt[:, o:o + 128], rhs=v_sb[:, h, sb, :],
                    start=(sb == 0), stop=(sb == nsb - 1),
                )
            rl = att.tile([128, 1], F32, tag="rl", name="rl", bufs=8)
            nc.vector.reciprocal(rl[:], yt[:, 64:65])
            if hl == 0:
                yn = att.tile([128, 128], BF16, tag="yn", name="yn", bufs=16)
                ynat_tiles[(p, tb)] = yn
            else:
                yn = ynat_tiles[(p, tb)]
            nc.vector.tensor_scalar_mul(yn[:, 64 * hl:64 * hl + 64], yt[:, 0:64], rl[:])
            if hl == 1:
                if pe_transpose is not None:
                    # latency-critical tail: PE transpose via identity + ACT
                    # evac beats the ~2.3us DMA-transpose round trip
                    trp = pe_transpose.tile([128, 1024], BF16, tag="o", name="trp")
                    nc.tensor.transpose(trp[:, 0:128], yn[:], ident_sb[:])
                    nc.vector.tensor_copy(y_sb[p][:, ts(tb, 128)], trp[:, 0:128])
                else:
                    nc.sync.dma_start_transpose(y_sb[p][:, ts(tb, 128)], yn[:])

        def emit_A(h, tcn, pe_transpose=None):
            for j in range(4):
                emit_A_j(h, tcn, j, pe_transpose=pe_transpose)

        def emit_P_tb(tb, ob, ps_o, evac_act=False, alt_y=False, split_dma=False):
            ot = ob.tile([128, 1024], BF16, tag="ot", name="ot", bufs=3)
            for cc in range(2):
                if alt_y and cc == 1:
                    # borrow the (idle) AV psum ring to double effective depth
                    op = ps_att.tile([128, 512], F32, tag="y", name="opsy")
                else:
                    op = ps_o.tile([128, 512], F32, tag="o", name="ops")
                for cl in range(2):
                    nc.tensor.matmul(
                        op[:], lhsT=y_sb[cl][:, ts(tb, 128)],
                        rhs=wo_sb[:, cl, ts(cc, 512)],
                        start=(cl == 0), stop=(cl == 1),
                    )
                if evac_act and cc == 1:
                    # tail region: split evacs DVE/ACT so neither saturates
                    nc.scalar.copy(ot[:, ts(cc, 512)], op[:])
                else:
                    nc.vector.tensor_copy(ot[:, ts(cc, 512)], op[:])
                if split_dma:
                    nc.sync.dma_start(out[tb][:, ts(cc, 512)], ot[:, ts(cc, 512)])
            if not split_dma:
                nc.sync.dma_start(out[tb], ot[:])

        # ---- emission schedule ----
        S = emit_S_pair
        emit_QK(0, "q", 0); emit_QK(0, "k", 0)
        S(0, 0, 0); S(0, 0, 1)
        emit_QK(0, "q", 1); emit_QK(0, "k", 1)
        S(0, 1, 0); S(0, 1, 1); S(0, 1, 2); S(0, 1, 3)
        S(1, 0, 0); S(1, 0, 1)
        emit_QK(0, "q", 2); emit_QK(0, "k", 2)
        S(1, 1, 0); S(1, 1, 1); S(1, 1, 2); S(1, 1, 3)
        emit_V(0); emit_V(1)
        emit_QK(0, "q", 3); emit_QK(0, "k", 3)
        emit_V(2); emit_V(3)
        emit_A(0, 0)
        for i in range(6):
            S(0, 2, i)
        emit_QK(1, "q", 0); emit_QK(1, "k", 0)
        emit_A(1, 0)
        for i in range(8):
            S(0, 3, i)
        emit_QK(1, "q", 1); emit_QK(1, "k", 1)
        for i in range(6):
            S(1, 2, i)
        emit_V(4); emit_V(5); emit_V(6); emit_V(7)
        emit_QK(1, "q", 2); emit_QK(1, "k", 2)
        for i in range(8):
            S(1, 3, i)
        emit_A(0, 1)
        emit_QK(1, "q", 3); emit_QK(1, "k", 3)
        S(2, 0, 0); S(2, 0, 1); S(3, 0, 0); S(3, 0, 1)
        emit_A(1, 1)
        emit_V(8); emit_V(9); emit_V(10); emit_V(11)
        for i in range(4):
            S(2, 1, i)
        for i in range(4):
            S(3, 1, i)
        emit_V(12); emit_V(13); emit_V(14); emit_V(15)

        # BC done: release its PSUM banks + SBUF, open projection pools
        bc_ctx.close()
        pr_ctx = ExitStack()
        ps_o = pr_ctx.enter_context(tc.tile_pool(name="ps_o", bufs=2, space="PSUM"))
        ob = pr_ctx.enter_context(tc.tile_pool(name="ob", bufs=1))

        def P(tcn):
            for tb in range(4 * tcn, 4 * tcn + 4):
                emit_P_tb(tb, ob, ps_o)

        emit_A_j(2, 0, 0); emit_A_j(2, 0, 1); S(2, 2, 0)
        emit_A_j(2, 0, 2); emit_A_j(2, 0, 3); S(2, 2, 1)
        emit_A_j(3, 0, 0); emit_A_j(3, 0, 1); S(2, 2, 2)
        emit_A_j(3, 0, 2); emit_A_j(3, 0, 3); S(2, 2, 3)
        emit_A_j(0, 3, 0); S(2, 2, 4); emit_A_j(0, 3, 1); S(2, 2, 5)
        emit_A_j(0, 3, 2); emit_A_j(0, 3, 3)
        P(0)
        emit_A_j(2, 1, 0); emit_A_j(2, 1, 1); S(3, 2, 0)
        emit_A_j(2, 1, 2); emit_A_j(2, 1, 3); S(3, 2, 1)
        emit_A_j(3, 1, 0); emit_A_j(3, 1, 1); S(3, 2, 2)
        emit_A_j(3, 1, 2); emit_A_j(3, 1, 3); S(3, 2, 3)
        emit_A_j(1, 3, 0); S(3, 2, 4); emit_A_j(1, 3, 1); S(3, 2, 5)
        emit_A_j(1, 3, 2); emit_A_j(1, 3, 3)
        emit_A_j(0, 2, 0); emit_A_j(0, 2, 1); emit_A_j(0, 2, 2); emit_A_j(0, 2, 3)
        emit_A_j(1, 2, 0); emit_A_j(1, 2, 1); S(2, 3, 0)
        emit_A_j(1, 2, 2); emit_A_j(1, 2, 3); S(2, 3, 1)
        S(2, 3, 2); S(2, 3, 3)
        S(3, 3, 0); S(3, 3, 1)
        S(2, 3, 4); S(2, 3, 5); S(2, 3, 6); S(2, 3, 7)
        S(3, 3, 2); S(3, 3, 3)
        P(1)
        S(3, 3, 4)
        emit_A_j(2, 2, 0); emit_A_j(2, 2, 1)
        S(3, 3, 5)
        emit_A_j(2, 2, 2); emit_A_j(2, 2, 3)
        S(3, 3, 6)
        emit_A_j(3, 2, 0); emit_A_j(3, 2, 1)
        S(3, 3, 7)
        emit_A_j(3, 2, 2); emit_A_j(3, 2, 3)
        emit_A_j(2, 3, 0); emit_P_tb(8, ob, ps_o, evac_act=True)
        emit_A_j(2, 3, 1); emit_P_tb(9, ob, ps_o, evac_act=True)
        emit_A_j(2, 3, 2); emit_P_tb(10, ob, ps_o, evac_act=True)
        emit_A_j(2, 3, 3); emit_P_tb(11, ob, ps_o, evac_act=True)
        emit_A_j(3, 3, 0, pe_transpose=ps_o)
        emit_A_j(3, 3, 1, pe_transpose=ps_o)
        emit_A_j(3, 3, 2, pe_transpose=ps_o)
        emit_A_j(3, 3, 3, pe_transpose=ps_o)
        emit_P_tb(12, ob, ps_o, evac_act=True, alt_y=True)
        emit_P_tb(13, ob, ps_o, evac_act=True, alt_y=True)
        emit_P_tb(14, ob, ps_o, evac_act=True, alt_y=True)
        emit_P_tb(15, ob, ps_o, evac_act=True, alt_y=True, split_dma=True)
        pr_ctx.close()

    nc.compile()
    return nc


_NC = None


def _get_nc():
    global _NC
    if _NC is None:
        _NC = build_nc()
    return _NC


def _mask_arr():
    p = np.arange(128)[:, None]
    f = np.arange(128)[None, :]
    tri = (p <= f).astype(np.float32)
    return np.ascontiguousarray(np.concatenate([tri, tri], axis=1).astype(BF))


def make_in_maps(x, w_q, w_k, w_v, w_o):
    x = np.asarray(x, dtype=np.float32)
    w_q = np.asarray(w_q, dtype=np.float32)
    w_k = np.asarray(w_k, dtype=np.float32)
    w_v = np.asarray(w_v, dtype=np.float32)
    w_o = np.asarray(w_o, dtype=np.float32)
    in_maps = []
    for c in range(NCORES):
        b, g = c // 4, c % 4
        hs = [4 * g + i for i in range(HPC)]
        xT = np.ascontiguousarray(x[b].T).reshape(CK, 128, T)
        wq_a = np.stack([
            np.concatenate([w_q[hs[2 * p]], w_q[hs[2 * p + 1]]], axis=1)
            .reshape(CK, 128, 128).transpose(1, 0, 2)
            for p in range(2)
        ])
        wk_a = np.stack([
            np.concatenate([w_k[hs[2 * p]], w_k[hs[2 * p + 1]]], axis=1)
            .reshape(CK, 128, 128).transpose(1, 0, 2)
            for p in range(2)
        ])
        wv_a = np.concatenate([w_v[h] for h in hs], axis=1).reshape(CK, 128, 256).transpose(1, 0, 2)
        wo_a = w_o[256 * g:256 * (g + 1)].reshape(2, 128, C)
        in_maps.append(dict(
            mask=_mask_arr(),
            ident=np.eye(128, dtype=BF),
            xT=np.ascontiguousarray(xT.astype(BF)),
            wq=np.ascontiguousarray(wq_a.astype(BF)),
            wk=np.ascontiguousarray(wk_a.astype(BF)),
            wv=np.ascontiguousarray(wv_a.astype(BF)),
            wo=np.ascontiguousarray(wo_a.astype(BF)),
        ))
    return in_maps


def gather_out(results):
    acc = [np.zeros((T, C), np.float64) for _ in range(B)]
    for c in range(NCORES):
        acc[c // 4] += results[c]["out"].reshape(T, C).astype(np.float64)
    return np.stack([a.astype(np.float32) for a in acc])


def run(x, w_q, w_k, w_v, w_o, trace=False, **spmd_kwargs):
    nc = _get_nc()
    in_maps = make_in_maps(x, w_q, w_k, w_v, w_o)
    res = run_bass_kernel_spmd(nc, in_maps, list(range(NCORES)), trace=trace,
                               **spmd_kwargs)
    return gather_out(res.results), res


def kernel(x, w_q, w_k, w_v, w_o):
    out, _ = run(x, w_q, w_k, w_v, w_o)
    return np.asarray(out, dtype=np.float32)
